# revision 24
# baseline (speedup 1.0000x reference)
"""Trainium2 Bass kernel for nn_Attention_78048145703090 (sparse_attention).

Math: the reference's [N,N] attention is rank-1 structured: logits[n,m] =
w_n * s_m with w_n = scale*exp(1-dist_n) depending only on the grid position n
and s_m = (wk^T q_center) . x_m. Additionally |w_n * s_m| <= 0.17 for all but
the 8 center-most distance classes, so exp(w_n s_m) is replaced by a degree-3
Taylor polynomial in t = s/S there, while the 8 "near" classes get exact exp
columns. The whole softmax+V+proj pipeline then reduces to:

  yt[c',j]  = sum_m x[m,c'] * phi_j(m)      phi = [1, t..t^3, exp(a_j t) x8]
  m_j       = sum_m phi_j(m)                (den coefficients)
  m23[j,c]  = sum_c' [yt;m][c',j] W2aug[c',c]  (W2aug folds wv/wp/bv/bp)
  out65[n,] = sum_j CT[j,n] * m23[j,:]      CT = compile-time Vandermonde/1-hot
  out[n,:]  = out65[n,0:64] / out65[n,64]

so there is no [N,N] attention, no 457-wide exp sweep, and no one-hot gather:
the final expansion is 8 quad matmuls with a 128KB compile-time bf16 constant
(block-diagonal moving operand covers 4 row-chunks per matmul).

x is shipped bf16 (host downcast: halves the input stream, feeds the
single-pass bf16 yt matmuls directly), in four quarter DMAs that pipeline
with the s-phase (DVE mul+reduce per quarter, Pool powers/exp-args, ACT exp).
u = (wk^T q_center)/S is folded on the host and shipped pre-broadcast.

Sharding: data-parallel over B=8 across the 8 cores (one sample per core).
"""

import sys

sys.path.insert(0, "/opt/trn_rl_repo")

import numpy as np

import concourse.bacc as bacc
import concourse.mybir as mybir
import concourse.tile as tile


def _install_profile_hook():
    """This image's antenv lacks axon_hooks; reconstruct it so
    run_bass_kernel_spmd(trace=True) can capture NTFF profiles."""
    import types

    try:
        import antenv.axon_hooks  # noqa: F401

        return
    except ImportError:
        pass
    try:
        import antenv

        m = types.ModuleType("antenv.axon_hooks")
        state = {"hook": None}
        m.set_axon_ntff_profile_hook = lambda h: state.__setitem__("hook", h)
        m.get_axon_ntff_profile_hook = lambda: state["hook"]
        sys.modules["antenv.axon_hooks"] = m
        antenv.axon_hooks = m
        from trn_agent_boot.trn_boot import _ntff_profile_via_ctypes

        m.set_axon_ntff_profile_hook(
            _ntff_profile_via_ctypes("/opt/axon/libaxon_pjrt.so")
        )
    except Exception:
        pass


_install_profile_hook()

from concourse.bass_utils import run_bass_kernel_spmd

B, H, W, C = 8, 64, 64, 64
N = H * W  # 4096
P = 128
NCH = N // P  # 32 chunks; n = p*32 + i
QSPLIT = [0, 10, 18, 26, 32]  # uneven s-phase quarters (small last quarter)
CENTER = (H // 2) * W + (W // 2)  # 2080
SCALE = float(C) ** -0.5
F32 = mybir.dt.float32
BF16 = mybir.dt.bfloat16

S = 16.0  # t = s / S normalization (folded into u host-side)
K = 3  # Taylor order: powers t^1..t^K
NEAR = 8  # exact-exp distance classes (d2 <= 10); max far |w*s| ~ 0.17
NCOL = 1 + K + NEAR  # 12 phi columns
NCOLP = 32  # padded phi-block stride (compute engines need 32-aligned bases)
QB = 4  # chunks per final quad matmul
NQ = NCH // QB  # 8 quad matmuls
QW = QB * (C + 1)  # 260 moving cols per quad
XW = NCH * C  # 2048 flat x columns (chunk i at i*64)

# ---- compile-time constants from the distance grid ----
_yy, _xx = np.mgrid[0:H, 0:W]
_d2 = ((_yy - H // 2) ** 2 + (_xx - W // 2) ** 2).reshape(-1)
_uniq, _g = np.unique(_d2, return_inverse=True)
_wt = SCALE * np.exp(1.0 - np.sqrt(_uniq.astype(np.float64)))
_a = _wt * S  # exp/Taylor argument scale applied to t
A_NEAR = [float(v) for v in _a[:NEAR]]

# CT [NCOL, N]: far n -> Vandermonde row in a_{g(n)}; near n -> one-hot exp col
_CT = np.zeros((NCOL, N), np.float64)
for _n in range(N):
    _u = _g[_n]
    if _u < NEAR:
        _CT[1 + K + _u, _n] = 1.0
    else:
        _fact = 1.0
        for _k in range(K + 1):
            _CT[_k, _n] = _fact
            _fact = _fact * _a[_u] / (_k + 1)

import ml_dtypes

# quad-packed stationary: ct4[b*NCOLP+j, q*P+p] = CT[j, p*NCH + (QB*q+b)]
_CT4 = np.zeros((QB * NCOLP, NQ * P), np.float64)
for _q in range(NQ):
    for _b in range(QB):
        _i = QB * _q + _b
        _CT4[_b * NCOLP : _b * NCOLP + NCOL, _q * P : (_q + 1) * P] = _CT[:, _i::NCH]
CT4 = np.ascontiguousarray(_CT4.astype(ml_dtypes.bfloat16))


def _view(ap, offset, dims):
    return type(ap)(tensor=ap.tensor, offset=offset, ap=dims)


def build_nc():
    nc = bacc.Bacc("TRN2", target_bir_lowering=False, debug=False, num_devices=B)
    # xub: host-packed [ubc | x in (p,i,c) order] so one DMA stream carries
    # both and every transfer is fully contiguous on both sides
    xub = nc.dram_tensor("xub", [P, C + XW], BF16, kind="ExternalInput")
    w2aug = nc.dram_tensor("w2aug", [C + 1, C + 1], BF16, kind="ExternalInput")
    ct4 = nc.dram_tensor("ct4", [QB * NCOLP, NQ * P], BF16, kind="ExternalInput")
    out = nc.dram_tensor("out", [N, C], F32, kind="ExternalOutput")

    xuv = xub.ap()
    ov = out.ap().rearrange("(p i) c -> p i c", p=P)

    with tile.TileContext(nc) as tc:
        with (
            tc.tile_pool(name="consts", bufs=1) as consts,
            tc.tile_pool(name="sb", bufs=1) as sb,
            tc.tile_pool(name="ps_small", bufs=2, space="PSUM") as ps_small,
            tc.tile_pool(name="ps_yt", bufs=1, space="PSUM") as ps_yt,
            tc.tile_pool(name="ps_o", bufs=5, space="PSUM") as ps_o,
        ):
            mov56 = sb.tile([QB * NCOLP, QW], BF16)
            nc.gpsimd.memset(mov56[:], 0.0)
            ones_col = consts.tile([P, 1], F32)
            nc.vector.memset(ones_col[:], 1.0)
            yt_rep = sb.tile([C + 1, QB * NCOLP], BF16)
            nc.vector.memset(yt_rep[:], 0.0)

            xub_sb = sb.tile([P, C + XW], BF16)
            ubc_sb = xub_sb[:, 0:C]
            abc_sb = consts.tile([P, NEAR], BF16)
            for j in range(NEAR):
                nc.gpsimd.memset(abc_sb[:, j : j + 1], A_NEAR[j])
            w2aug_sb = consts.tile([C + 1, C + 1], BF16)
            ct4_sb = consts.tile([QB * NCOLP, NQ * P], BF16)

            nc.sync.dma_start(
                out=xub_sb[:, 0 : C + QSPLIT[1] * C],
                in_=xuv[:, 0 : C + QSPLIT[1] * C],
            )
            for k in range(1, 4):
                nc.sync.dma_start(
                    out=xub_sb[:, C + QSPLIT[k] * C : C + QSPLIT[k + 1] * C],
                    in_=xuv[:, C + QSPLIT[k] * C : C + QSPLIT[k + 1] * C],
                )
            nc.sync.dma_start(out=w2aug_sb[:], in_=w2aug[:])
            nc.sync.dma_start(out=ct4_sb[:], in_=ct4[:])

            # phi [p, j, i] bf16: col 0 = ones, 1..K = t^k, K+1.. = exp(a_j t)
            phi = sb.tile([P, NCOL, NCH], BF16)
            nc.gpsimd.memset(phi[:, 0, :], 1.0)
            t_sb = sb.tile([P, NCH], F32)
            zarg = sb.tile([P, NEAR, NCH], F32)
            xu = sb.tile([P, NCH, C], F32)
            phisum = sb.tile([P, NCOL], F32)
            m_ps = ps_small.tile([1, NCOL], F32, tag="m")
            ubc_ap = ubc_sb
            t_ap = t_sb[:]
            a_in = abc_sb[:]
            yt_ps = ps_yt.tile([C, NCOLP], F32)

            def s_mul(k4, eng):
                i0, i1 = QSPLIT[k4], QSPLIT[k4 + 1]
                xin = xub_sb[:, C + i0 * C : C + i1 * C].rearrange(
                    "p (i c) -> p i c", c=C
                )
                ubc_h = _view(
                    ubc_ap, ubc_ap.offset, [ubc_ap.ap[0], [0, i1 - i0], ubc_ap.ap[1]]
                )
                eng.tensor_mul(xu[:, i0:i1, :], xin, ubc_h)

            def s_red(k4):
                i0, i1 = QSPLIT[k4], QSPLIT[k4 + 1]
                nc.vector.tensor_reduce(
                    out=t_sb[:, i0:i1],
                    in_=xu[:, i0:i1, :],
                    op=mybir.AluOpType.add,
                    axis=mybir.AxisListType.X,
                )

            def phi_ops(k4, eng):
                i0, i1 = QSPLIT[k4], QSPLIT[k4 + 1]
                QH = i1 - i0
                eng.tensor_copy(out=phi[:, 1, i0:i1], in_=t_sb[:, i0:i1])
                for k in range(2, K + 1):
                    eng.tensor_mul(
                        phi[:, k, i0:i1], phi[:, k - 1, i0:i1], phi[:, 1, i0:i1]
                    )
                p1 = phi[:, 1, :]
                t_b = _view(p1, p1.offset + i0, [p1.ap[0], [0, NEAR], [1, QH]])
                a_b = _view(a_in, a_in.offset, [a_in.ap[0], [1, NEAR], [0, QH]])
                eng.tensor_mul(zarg[:, :, i0:i1], t_b, a_b)

            def exp_yt(k4):
                i0, i1 = QSPLIT[k4], QSPLIT[k4 + 1]
                nc.scalar.activation(
                    out=phi[:, 1 + K : 1 + K + NEAR, i0:i1],
                    in_=zarg[:, :, i0:i1],
                    func=mybir.ActivationFunctionType.Exp,
                )
                for i in range(i0, i1):
                    nc.tensor.matmul(
                        yt_ps[:, 0:NCOL],
                        xub_sb[:, C + i * C : C + (i + 1) * C],
                        phi[:, :, i],
                        start=(i == 0),
                        stop=(i == NCH - 1),
                    )

            # schedule: Pool runs phi for q0-q2 plus the q3 multiply (hoisted
            # before q2's phi so it fires as soon as the last x DMA lands);
            # DVE runs all reduces, the q0-q2 multiplies, and q3's phi chain
            s_mul(0, nc.vector); s_red(0); phi_ops(0, nc.gpsimd); exp_yt(0)
            s_mul(1, nc.vector); s_red(1); phi_ops(1, nc.gpsimd); exp_yt(1)
            s_mul(3, nc.gpsimd)
            s_mul(2, nc.vector); s_red(2); phi_ops(2, nc.gpsimd); exp_yt(2)
            s_red(3); phi_ops(3, nc.vector); exp_yt(3)

            # den coefficients m_j = sum_m phi_j(m), split so the poly part
            # reduces before the last exp lands
            NP1 = 1 + K
            nc.vector.tensor_reduce(
                out=phisum[:, 0:NP1],
                in_=phi[:, 0:NP1, :],
                op=mybir.AluOpType.add,
                axis=mybir.AxisListType.X,
            )
            nc.vector.tensor_reduce(
                out=phisum[:, NP1:NCOL],
                in_=phi[:, NP1:NCOL, :],
                op=mybir.AluOpType.add,
                axis=mybir.AxisListType.X,
            )
            nc.tensor.matmul(
                m_ps[:, 0:NP1], ones_col[:], phisum[:, 0:NP1],
                start=True, stop=True,
            )
            nc.tensor.matmul(
                m_ps[:, NP1:NCOL], ones_col[:], phisum[:, NP1:NCOL],
                start=True, stop=True,
            )

            # replicate yt's columns (+ the m row) into the four 32-aligned
            # blocks so FK emits all diagonal blocks on their own partitions
            yt_src = yt_ps[0:C, :]
            rep_in = _view(yt_src, yt_src.offset, [yt_src.ap[0], [0, QB], [1, NCOL]])
            yr_ap = yt_rep[:]
            rep_out = _view(
                yr_ap, yr_ap.offset, [[yr_ap.ap[0][0], C], [NCOLP, QB], [1, NCOL]]
            )
            nc.vector.tensor_copy(out=rep_out, in_=rep_in)
            m_ap = m_ps[:]
            m_in = _view(m_ap, m_ap.offset, [m_ap.ap[0], [0, QB], [1, NCOL]])
            mr_ap = yt_rep[C : C + 1, :]
            m_out = _view(mr_ap, mr_ap.offset, [mr_ap.ap[0], [NCOLP, QB], [1, NCOL]])
            nc.vector.tensor_copy(out=m_out, in_=m_in)

            # m23[j, c] = sum_c' yt[c', j] W2aug[c', c], quad-replicated
            m56_ps = ps_small.tile([QB * NCOLP, C + 1], F32, tag="m")
            nc.tensor.matmul(
                m56_ps[:], yt_rep[:], w2aug_sb[:], start=True, stop=True
            )
            for b in range(QB):
                sr = m56_ps[b * NCOLP : b * NCOLP + NCOL, :]
                ds = mov56[
                    b * NCOLP : b * NCOLP + NCOL, b * (C + 1) : (b + 1) * (C + 1)
                ]
                if b % 2 == 0:
                    nc.vector.tensor_copy(out=ds, in_=sr)
                else:
                    nc.scalar.copy(out=ds, in_=sr)

            # final: out65 for 4 chunks per matmul; scale by 1/den; store.
            # Drain 3-way: quads 0-3 ACT-copy to SBUF then Pool wide-multiply,
            # quads 4-7 DVE-direct wide-multiply; recips on DVE throughout.
            r_sb = sb.tile([P, NCH], F32)
            o_sb = sb.tile([P, NCH, C], F32)
            o_cp = sb.tile([P, QB, QW], F32)
            for q in range(NQ):
                o_ps = ps_o.tile([P, QW], F32)
                nc.tensor.matmul(
                    o_ps[:], ct4_sb[:, q * P : (q + 1) * P], mov56[:],
                    start=True, stop=True,
                )
                o_ap = o_ps[:]
                nc.vector.reciprocal(
                    out=r_sb[:, q * QB : (q + 1) * QB],
                    in_=o_ap[:, C : QW : C + 1],
                )
                r_ap = r_sb[:, q * QB : (q + 1) * QB]
                r_b = _view(r_ap, r_ap.offset, [r_ap.ap[0], [1, QB], [0, C]])
                if q < 4:
                    nc.scalar.copy(out=o_cp[:, q, :], in_=o_ps[:])
                    cp_ap = o_cp[:, q, :]
                    num_v = _view(
                        cp_ap, cp_ap.offset, [cp_ap.ap[0], [C + 1, QB], [1, C]]
                    )
                    nc.gpsimd.tensor_mul(
                        o_sb[:, q * QB : (q + 1) * QB, :], num_v, r_b
                    )
                else:
                    num_v = _view(
                        o_ap, o_ap.offset, [o_ap.ap[0], [C + 1, QB], [1, C]]
                    )
                    nc.vector.tensor_mul(
                        o_sb[:, q * QB : (q + 1) * QB, :], num_v, r_b
                    )
                if q in (4, 7):
                    lo = {4: 0, 7: 20}[q]
                    hi = (q + 1) * QB
                    nc.sync.dma_start(
                        out=ov[:, lo:hi, :], in_=o_sb[:, lo:hi, :]
                    )

    nc.compile()
    return nc


_nc_cache = None


def _get_nc():
    global _nc_cache
    if _nc_cache is None:
        _nc_cache = build_nc()
    return _nc_cache


def make_in_maps(x, wq, bq, wk, bk, wv, bv, wp, bp):
    f = lambda a: np.asarray(a, dtype=np.float32)
    x = f(x)
    wq, bq, wk, bk, wv, bv, wp, bp = map(f, (wq, bq, wk, bk, wv, bv, wp, bp))
    w2aug_h = np.zeros((C + 1, C + 1), np.float32)
    w2aug_h[0:C, 0:C] = wv.T @ wp.T
    w2aug_h[C, 0:C] = wp @ bv + bp
    w2aug_h[C, C] = 1.0
    shared = {
        "w2aug": np.ascontiguousarray(w2aug_h.astype(ml_dtypes.bfloat16)),
        "ct4": CT4,
    }
    maps = []
    for b in range(B):
        xf = x[b].reshape(N, C)
        u_row = (((wq @ xf[CENTER] + bq) @ wk) / np.float32(S)).astype(np.float32)
        xub_h = np.empty((P, C + XW), ml_dtypes.bfloat16)
        xub_h[:, 0:C] = u_row.astype(ml_dtypes.bfloat16)[None, :]
        xub_h[:, C:] = (
            xf.reshape(P, NCH, C).astype(ml_dtypes.bfloat16).reshape(P, XW)
        )
        maps.append({"xub": np.ascontiguousarray(xub_h), **shared})
    return maps


def kernel_with_results(trace=False, **inputs):
    in_maps = make_in_maps(**inputs)
    nc = _get_nc()
    res = run_bass_kernel_spmd(nc, in_maps, core_ids=list(range(B)), trace=trace)
    out = np.stack([r["out"] for r in res.results], 0).reshape(B, H, W, C)
    return out, res


def kernel(**inputs):
    out, _ = kernel_with_results(**inputs)
    return out
